# revision 17
# baseline (speedup 1.0000x reference)
"""Trainium2 Bass kernel for nn_Point2Mask (retrieval_knn).

Full inputs in, full output out. Sharding: data-parallel over the K=8
instance axis — core c handles instance c (its 1024 points) across all
M=4 views. Each core rasterizes the 48x48 ball-query mask for its 4
(instance, view) pairs.

Device algorithm per core:
  - project points to 2D per view (PE matmul), normalize via min/max
  - per-point feature top2-gap df (segmented reduces)
  - broadcast coords/df rows to 128 partitions via ones-matmul (PE)
  - main loop over 4 views x 18 pixel-tiles of [128 pixels, 1024 points]:
      ACT: dx2 = Square(px - gx), dy2 = Square(py - gy), t4 = 4 - dy2
      DVE: within = dx2 < t4 (bf16), inclusive cumsum (tensor_tensor_scan),
           sel = (csum <= 16) & within, fsd = sum(sel * df) via accum_out,
           count = min(csum_last, 16)
  - tail: p1 = sigmoid(fsd / max(count,1)) * 255, zeroed where count==0
"""
import numpy as np

import concourse.bass as bass
import concourse.bacc as bacc
import concourse.mybir as mybir
from concourse import tile
from concourse.bass_utils import run_bass_kernel_spmd

K = 8
M = 4
N = 8192
NPER = N // K          # 1024 points per instance
C = 20
RES = 48
S = RES * RES          # 2304 pixels
NT = S // 128          # 18 pixel tiles
NCHUNK = NPER // 128   # 8 point chunks
R2 = 4.0               # radius^2
NS = 16.0              # nsample

F32 = mybir.dt.float32
BF16 = mybir.dt.bfloat16
A = mybir.AluOpType
AF = mybir.ActivationFunctionType


def build_program():
    nc = bacc.Bacc("TRN2", target_bir_lowering=False, debug=True)
    d_xyzT = nc.dram_tensor("xyzT", [3, NPER], F32, kind="ExternalInput")
    d_feats = nc.dram_tensor("featsR", [128, NCHUNK * C], F32, kind="ExternalInput")
    d_wmat = nc.dram_tensor("wmat", [3, 2 * M], F32, kind="ExternalInput")
    d_bias8 = nc.dram_tensor("bias8", [2 * M, 1], F32, kind="ExternalInput")
    d_ngx = nc.dram_tensor("ngx", [128, NT], F32, kind="ExternalInput")
    d_ngy = nc.dram_tensor("ngy", [128, NT], F32, kind="ExternalInput")
    d_ones1 = nc.dram_tensor("ones1", [1, 128], F32, kind="ExternalInput")
    d_eye8 = nc.dram_tensor("eye8rep", [NCHUNK, NCHUNK * 128], F32, kind="ExternalInput")
    d_wrep = nc.dram_tensor("wrep", [3, 2 * M * 128], F32, kind="ExternalInput")
    d_ident = nc.dram_tensor("ident", [128, 128], F32, kind="ExternalInput")
    d_out = nc.dram_tensor("out", [M, S], F32, kind="ExternalOutput")

    with tile.TileContext(nc) as tc:
        with (
            tc.tile_pool(name="cst", bufs=1) as cst,
            tc.tile_pool(name="prep", bufs=1) as prep,
            tc.tile_pool(name="big", bufs=1) as big,
            tc.tile_pool(name="loop", bufs=2) as lp,
            tc.tile_pool(name="ps", bufs=1, space="PSUM") as ps,
            tc.tile_pool(name="ps2", bufs=2, space="PSUM") as ps2,
        ):
            # ---- load constants/inputs ----
            xyzT = cst.tile([3, NPER], F32)
            feats = cst.tile([128, NCHUNK * C], F32)
            wmat = cst.tile([3, 2 * M], F32)
            bias8 = cst.tile([2 * M, 1], F32)
            ngx = cst.tile([128, NT], F32)
            ngy = cst.tile([128, NT], F32)
            ones1 = cst.tile([1, 128], F32)
            eye8 = cst.tile([NCHUNK, NCHUNK * 128], F32)
            wrep = cst.tile([3, 2 * M * 128], F32)
            ident = cst.tile([128, 128], F32)
            for t, d in [(xyzT, d_xyzT), (feats, d_feats), (wmat, d_wmat),
                         (bias8, d_bias8), (ngx, d_ngx), (ngy, d_ngy),
                         (ones1, d_ones1), (eye8, d_eye8), (wrep, d_wrep),
                         (ident, d_ident)]:
                nc.sync.dma_start(t[:], d[:])

            c4b = cst.tile([128, 1], F32)
            nc.vector.memset(c4b[:], R2)
            czero = cst.tile([128, 1], F32)
            nc.vector.memset(czero[:], 0.0)

            # ---- df = top1 - top2 of features per point ----
            fview = feats[:].rearrange("p (c k) -> p c k", c=NCHUNK)
            m1 = prep.tile([128, NCHUNK], F32)
            nc.vector.tensor_reduce(
                m1[:].rearrange("p (c o) -> p c o", o=1), fview,
                axis=mybir.AxisListType.X, op=A.max)
            h = prep.tile([128, NCHUNK * C], F32)
            for c in range(NCHUNK):
                nc.vector.tensor_scalar(
                    h[:, c * C:(c + 1) * C], feats[:, c * C:(c + 1) * C],
                    m1[:, c:c + 1], None, op0=A.subtract)
            hmask = prep.tile([128, NCHUNK * C], F32)
            nc.vector.tensor_scalar(hmask[:], h[:], 0.0, None, op0=A.is_ge)
            hm = prep.tile([128, NCHUNK * C], F32)
            nc.vector.scalar_tensor_tensor(
                hm[:], hmask[:], -1e9, h[:], op0=A.mult, op1=A.add)
            m2n = prep.tile([128, NCHUNK], F32)
            nc.vector.tensor_reduce(
                m2n[:].rearrange("p (c o) -> p c o", o=1),
                hm[:].rearrange("p (c k) -> p c k", c=NCHUNK),
                axis=mybir.AxisListType.X, op=A.max)
            df8 = prep.tile([128, NCHUNK], F32)
            nc.vector.tensor_scalar_mul(df8[:], m2n[:], -1.0)

            # transpose df8 -> [NCHUNK, 128], then broadcast chunks via
            # one-hot matmuls: dfb[p, c*128+q] = dfrowS[c, q]
            dfrow_ps = ps.tile([NCHUNK, 128], F32, tag="pp")
            nc.tensor.transpose(dfrow_ps[:], df8[:], ident[:])
            dfrowS = prep.tile([NCHUNK, 128], F32)
            nc.scalar.copy(dfrowS[:], dfrow_ps[:])
            dfb_ps = ps.tile([128, NPER], F32, tag="pp")
            for c in range(NCHUNK):
                nc.tensor.matmul(dfb_ps[:, c * 128:(c + 1) * 128],
                                 eye8[:, c * 128:(c + 1) * 128], dfrowS[:],
                                 start=True, stop=True)
            dfb = big.tile([128, NPER], BF16)
            nc.scalar.copy(dfb[:], dfb_ps[:])

            # ---- projected coords stats (small [8, NPER] pipeline) ----
            raw_ps = ps.tile([2 * M, NPER], F32, tag="rawp")
            for q in range(0, NPER, 512):
                nc.tensor.matmul(raw_ps[:, q:q + 512], wmat[:],
                                 xyzT[:, q:q + 512], start=True, stop=True)
            raw = prep.tile([2 * M, NPER], F32)
            nc.scalar.activation(raw[:], raw_ps[:], AF.Identity,
                                 bias=bias8[:], scale=1.0)
            rmax = prep.tile([2 * M, 1], F32)
            rmin = prep.tile([2 * M, 1], F32)
            nc.vector.tensor_reduce(rmax[:], raw[:], axis=mybir.AxisListType.X, op=A.max)
            nc.vector.tensor_reduce(rmin[:], raw[:], axis=mybir.AxisListType.X, op=A.min)
            cen = prep.tile([2 * M, 1], F32)
            nc.vector.tensor_tensor(cen[:], rmax[:], rmin[:], op=A.add)
            nc.vector.tensor_scalar_mul(cen[:], cen[:], 0.5)
            sp = prep.tile([2 * M, 1], F32)
            nc.vector.tensor_tensor(sp[:], rmax[:], rmin[:], op=A.subtract)
            nc.vector.tensor_scalar_max(sp[:], sp[:], 1e-5)
            rec = prep.tile([2 * M, 1], F32)
            nc.vector.reciprocal(rec[:], sp[:])
            a8 = prep.tile([2 * M, 1], F32)
            nc.vector.tensor_scalar_mul(a8[:], rec[:], 38.4)
            # b2 = a8*(bias8 - cen) + 24
            u8 = prep.tile([2 * M, 1], F32)
            nc.vector.tensor_tensor(u8[:], bias8[:], cen[:], op=A.subtract)
            b2 = prep.tile([2 * M, 1], F32)
            nc.vector.tensor_tensor(b2[:], a8[:], u8[:], op=A.mult)
            nc.vector.tensor_scalar_add(b2[:], b2[:], 24.0)

            # broadcast a8/b2 to [128, 16] via transpose + ones matmul
            abT_ps = ps.tile([1, 16], F32, tag="pp")
            nc.tensor.transpose(abT_ps[0:1, 0:8], a8[:], ident[0:8, 0:8])
            nc.tensor.transpose(abT_ps[0:1, 8:16], b2[:], ident[0:8, 0:8])
            abrow = prep.tile([1, 16], F32)
            nc.vector.tensor_copy(abrow[:], abT_ps[:])
            abB_ps = ps.tile([128, 16], F32, tag="pp")
            nc.tensor.matmul(abB_ps[:, 0:8], ones1[:], abrow[0:1, 0:8],
                             start=True, stop=True)
            nc.tensor.matmul(abB_ps[:, 8:16], ones1[:], abrow[0:1, 8:16],
                             start=True, stop=True)
            abB = prep.tile([128, 16], F32)
            nc.scalar.copy(abB[:], abB_ps[:])

            # ---- per-row broadcast coords: matmul with replicated weights,
            # normalization fused into the PSUM->SBUF copy ----
            rows = []
            for r in range(2 * M):
                pb_ps = ps2.tile([128, NPER], F32, tag="pb_ps")
                for q in range(0, NPER, 512):
                    nc.tensor.matmul(pb_ps[:, q:q + 512],
                                     wrep[:, r * 128:(r + 1) * 128],
                                     xyzT[:, q:q + 512], start=True, stop=True)
                prow = big.tile([128, NPER], F32, tag=f"prow{r}")
                nc.scalar.activation(prow[:], pb_ps[:], AF.Identity,
                                     bias=abB[:, 8 + r:9 + r], scale=abB[:, r:r + 1])
                rows.append(prow)

            # ---- main loop ----
            fsdcols = big.tile([128, M * NT], F32)
            cntcols = big.tile([128, M * NT], F32)
            junk = big.tile([128, NPER], BF16)
            for m in range(M):
                for t in range(NT):
                    col = m * NT + t
                    dx2 = lp.tile([128, NPER], F32, tag="dx2")
                    dy2 = lp.tile([128, NPER], F32, tag="dy2")
                    nc.scalar.activation(dx2[:], rows[2 * m][:], AF.Square,
                                         bias=ngx[:, t:t + 1], scale=1.0)
                    nc.scalar.activation(dy2[:], rows[2 * m + 1][:], AF.Square,
                                         bias=ngy[:, t:t + 1], scale=1.0)
                    t4 = lp.tile([128, NPER], F32, tag="t4")
                    nc.scalar.activation(t4[:], dy2[:], AF.Identity,
                                         bias=c4b[:], scale=-1.0)
                    w = lp.tile([128, NPER], BF16, tag="w")
                    nc.vector.scalar_tensor_tensor(
                        w[:], dx2[:], 1.0, t4[:], op0=A.mult, op1=A.is_lt)
                    satc = lp.tile([128, NPER], BF16, tag="satc")
                    nc.vector.tensor_tensor_scan(
                        satc[:], w[:], w[:], 0.0, op0=A.add, op1=A.bypass)
                    sel = lp.tile([128, NPER], BF16, tag="sel")
                    nc.vector.scalar_tensor_tensor(
                        sel[:], satc[:], NS, w[:], op0=A.is_le, op1=A.logical_and,
                        accum_out=cntcols[:, col:col + 1])
                    nc.vector.scalar_tensor_tensor(
                        junk[:], sel[:], 1.0, dfb[:], op0=A.mult, op1=A.mult,
                        accum_out=fsdcols[:, col:col + 1])

            # ---- tail ----
            occ = prep.tile([128, M * NT], F32)
            nc.vector.tensor_scalar_max(occ[:], cntcols[:], 1.0)
            rocc = prep.tile([128, M * NT], F32)
            nc.vector.reciprocal(rocc[:], occ[:])
            dlog = prep.tile([128, M * NT], F32)
            nc.vector.tensor_tensor(dlog[:], fsdcols[:], rocc[:], op=A.mult)
            p1 = prep.tile([128, M * NT], F32)
            nc.scalar.activation(p1[:], dlog[:], AF.Sigmoid, bias=czero[:], scale=1.0)
            nz = prep.tile([128, M * NT], F32)
            nc.vector.tensor_scalar(nz[:], cntcols[:], 0.0, None, op0=A.is_gt)
            outt = prep.tile([128, M * NT], F32)
            nc.vector.scalar_tensor_tensor(
                outt[:], p1[:], 255.0, nz[:], op0=A.mult, op1=A.mult)
            nc.sync.dma_start(
                d_out[:].rearrange("m (t p) -> p m t", p=128),
                outt[:].rearrange("p (m t) -> p m t", m=M))
    nc.compile()
    return nc


def host_inputs(xyz, features, theta, phi):
    """Build per-core input maps. xyz (1,N,3) f32, features (1,N,C) f32."""
    theta = np.asarray(theta, np.float32)
    phi = np.asarray(phi, np.float32)
    sint, cost = np.sin(theta), np.cos(theta)
    sinp, cosp = np.sin(phi), np.cos(phi)
    U = np.stack([-sint, cost, np.zeros_like(theta)], -1)      # (M,3)
    V = np.stack([cost * sinp, sint * sinp, cosp], -1)         # (M,3)
    center = np.stack([cost * cosp, sint * cosp, sinp], -1)    # (M,3)
    wmat = np.zeros((3, 2 * M), np.float32)
    bias8 = np.zeros((2 * M, 1), np.float32)
    for m in range(M):
        wmat[:, 2 * m] = U[m]
        wmat[:, 2 * m + 1] = V[m]
        bias8[2 * m, 0] = -np.dot(center[m], U[m])
        bias8[2 * m + 1, 0] = -np.dot(center[m], V[m])

    s = np.arange(S)
    p = s % 128
    t = s // 128
    ngx = np.zeros((128, NT), np.float32)
    ngy = np.zeros((128, NT), np.float32)
    ngx[p, t] = -(s // RES).astype(np.float32)
    ngy[p, t] = -(s % RES).astype(np.float32)

    ident = np.eye(128, dtype=np.float32)
    ones1 = np.ones((1, 128), np.float32)
    eye8rep = np.zeros((NCHUNK, NCHUNK * 128), np.float32)
    for c in range(NCHUNK):
        eye8rep[c, c * 128:(c + 1) * 128] = 1.0
    wrep = np.repeat(wmat, 128, axis=1)  # (3, 8*128), col r*128+p = wmat[:, r]

    xyz = np.asarray(xyz, np.float32)[0]          # (N,3)
    features = np.asarray(features, np.float32)[0]  # (N,C)
    in_maps = []
    for c in range(K):
        pts = xyz[c * NPER:(c + 1) * NPER]        # (1024,3)
        ft = features[c * NPER:(c + 1) * NPER]    # (1024,20)
        featsR = np.ascontiguousarray(
            ft.reshape(NCHUNK, 128, C).transpose(1, 0, 2).reshape(128, NCHUNK * C))
        in_maps.append({
            "xyzT": np.ascontiguousarray(pts.T),
            "featsR": featsR,
            "wmat": wmat, "bias8": bias8,
            "ngx": ngx, "ngy": ngy,
            "ones1": ones1, "eye8rep": eye8rep, "wrep": wrep, "ident": ident,
        })
    return in_maps


_CACHE = {}


def kernel(xyz, features, proposals, res, theta, phi):
    if "nc" not in _CACHE:
        _CACHE["nc"] = build_program()
    nc = _CACHE["nc"]
    in_maps = host_inputs(xyz, features, theta, phi)
    res_k = run_bass_kernel_spmd(nc, in_maps, list(range(K)))
    mask1 = np.stack([np.asarray(res_k.results[c]["out"]) for c in range(K)])
    mask1 = mask1.reshape(K * M, 1, RES, RES)
    out = np.broadcast_to(mask1, (K * M, 3, RES, RES))
    return np.ascontiguousarray(out.astype(np.float32))


# revision 21
# speedup vs baseline: 1.2070x; 1.2070x over previous
"""Trainium2 Bass kernel for nn_Point2Mask (retrieval_knn).

Full inputs in, full output out. Sharding: data-parallel over the K=8
instance axis — core c handles instance c (its 1024 points) across all
M=4 views. Each core rasterizes the 48x48 ball-query mask for its 4
(instance, view) pairs.

Device algorithm per core:
  - project points to 2D per view (PE matmul), normalize via min/max
  - per-point feature top2-gap df (segmented reduces)
  - broadcast coords/df rows to 128 partitions via ones-matmul (PE)
  - main loop over 4 views x 18 pixel-tiles of [128 pixels, 1024 points]:
      ACT: dx2 = Square(px - gx), dy2 = Square(py - gy), t4 = 4 - dy2
      DVE: within = dx2 < t4 (bf16), inclusive cumsum (tensor_tensor_scan),
           sel = (csum <= 16) & within, fsd = sum(sel * df) via accum_out,
           count = min(csum_last, 16)
  - tail: p1 = sigmoid(fsd / max(count,1)) * 255, zeroed where count==0
"""
import numpy as np

import concourse.bass as bass
import concourse.bacc as bacc
import concourse.mybir as mybir
from concourse import tile
from concourse.bass_utils import run_bass_kernel_spmd

K = 8
M = 4
N = 8192
NPER = N // K          # 1024 points per instance
C = 20
RES = 48
S = RES * RES          # 2304 pixels
NT = S // 128          # 18 pixel tiles
NCHUNK = NPER // 128   # 8 point chunks
R2 = 4.0               # radius^2
NS = 16.0              # nsample

F32 = mybir.dt.float32
BF16 = mybir.dt.bfloat16
A = mybir.AluOpType
AF = mybir.ActivationFunctionType


def build_program():
    nc = bacc.Bacc("TRN2", target_bir_lowering=False, debug=True)
    d_xyzT = nc.dram_tensor("xyzT", [3, NPER], F32, kind="ExternalInput")
    d_feats = nc.dram_tensor("featsR", [128, NCHUNK * C], F32, kind="ExternalInput")
    d_wmat = nc.dram_tensor("wmat", [3, 2 * M], F32, kind="ExternalInput")
    d_bias8 = nc.dram_tensor("bias8", [2 * M, 1], F32, kind="ExternalInput")
    d_ngx = nc.dram_tensor("ngx", [128, NT], F32, kind="ExternalInput")
    d_ngy = nc.dram_tensor("ngy", [128, NT], F32, kind="ExternalInput")
    d_ones1 = nc.dram_tensor("ones1", [1, 128], F32, kind="ExternalInput")
    d_eye8 = nc.dram_tensor("eye8rep", [NCHUNK, NCHUNK * 128], F32, kind="ExternalInput")
    d_wrep = nc.dram_tensor("wrep", [3, 2 * M * 128], F32, kind="ExternalInput")
    d_ident = nc.dram_tensor("ident", [128, 128], F32, kind="ExternalInput")
    d_out = nc.dram_tensor("out", [M, S], F32, kind="ExternalOutput")

    with tile.TileContext(nc) as tc:
        with (
            tc.tile_pool(name="cst", bufs=1) as cst,
            tc.tile_pool(name="prep", bufs=1) as prep,
            tc.tile_pool(name="big", bufs=1) as big,
            tc.tile_pool(name="loop", bufs=2) as lp,
            tc.tile_pool(name="ps", bufs=1, space="PSUM") as ps,
            tc.tile_pool(name="ps2", bufs=2, space="PSUM") as ps2,
        ):
            # ---- load constants/inputs ----
            xyzT = cst.tile([3, NPER], F32)
            feats = cst.tile([128, NCHUNK * C], F32)
            wmat = cst.tile([3, 2 * M], F32)
            bias8 = cst.tile([2 * M, 1], F32)
            ngx = cst.tile([128, NT], F32)
            ngy = cst.tile([128, NT], F32)
            ones1 = cst.tile([1, 128], F32)
            eye8 = cst.tile([NCHUNK, NCHUNK * 128], F32)
            wrep = cst.tile([3, 2 * M * 128], F32)
            ident = cst.tile([128, 128], F32)
            for t, d in [(xyzT, d_xyzT), (feats, d_feats), (wmat, d_wmat),
                         (bias8, d_bias8), (ngx, d_ngx), (ngy, d_ngy),
                         (ones1, d_ones1), (eye8, d_eye8), (wrep, d_wrep),
                         (ident, d_ident)]:
                nc.sync.dma_start(t[:], d[:])

            c4b = cst.tile([128, 1], F32)
            nc.vector.memset(c4b[:], R2)
            czero = cst.tile([128, 1], F32)
            nc.vector.memset(czero[:], 0.0)
            c16t = cst.tile([128, NPER], BF16)
            nc.vector.memset(c16t[:], NS)

            # ---- df = top1 - top2 of features per point ----
            fview = feats[:].rearrange("p (c k) -> p c k", c=NCHUNK)
            m1 = prep.tile([128, NCHUNK], F32)
            nc.vector.tensor_reduce(
                m1[:].rearrange("p (c o) -> p c o", o=1), fview,
                axis=mybir.AxisListType.X, op=A.max)
            h = prep.tile([128, NCHUNK * C], F32)
            for c in range(NCHUNK):
                nc.vector.tensor_scalar(
                    h[:, c * C:(c + 1) * C], feats[:, c * C:(c + 1) * C],
                    m1[:, c:c + 1], None, op0=A.subtract)
            hmask = prep.tile([128, NCHUNK * C], F32)
            nc.vector.tensor_scalar(hmask[:], h[:], 0.0, None, op0=A.is_ge)
            hm = prep.tile([128, NCHUNK * C], F32)
            nc.vector.scalar_tensor_tensor(
                hm[:], hmask[:], -1e9, h[:], op0=A.mult, op1=A.add)
            m2n = prep.tile([128, NCHUNK], F32)
            nc.vector.tensor_reduce(
                m2n[:].rearrange("p (c o) -> p c o", o=1),
                hm[:].rearrange("p (c k) -> p c k", c=NCHUNK),
                axis=mybir.AxisListType.X, op=A.max)
            df8 = prep.tile([128, NCHUNK], F32)
            nc.vector.tensor_scalar_mul(df8[:], m2n[:], -1.0)

            # transpose df8 -> [NCHUNK, 128], then broadcast chunks via
            # one-hot matmuls: dfb[p, c*128+q] = dfrowS[c, q]
            dfrow_ps = ps.tile([NCHUNK, 128], F32, tag="pp")
            nc.tensor.transpose(dfrow_ps[:], df8[:], ident[:])
            dfrowS = prep.tile([NCHUNK, 128], F32)
            nc.scalar.copy(dfrowS[:], dfrow_ps[:])
            dfb_ps = ps.tile([128, NPER], F32, tag="pp")
            for c in range(NCHUNK):
                nc.tensor.matmul(dfb_ps[:, c * 128:(c + 1) * 128],
                                 eye8[:, c * 128:(c + 1) * 128], dfrowS[:],
                                 start=True, stop=True)
            dfbf = big.tile([128, NPER], F32)
            nc.scalar.copy(dfbf[:], dfb_ps[:])
            eab = big.tile([128, NPER], F32)
            nc.vector.tensor_tensor(eab[:, 0:NPER - 1], dfbf[:, 0:NPER - 1],
                                    dfbf[:, 1:NPER], op=A.subtract)
            nc.vector.tensor_copy(eab[:, NPER - 1:NPER], dfbf[:, NPER - 1:NPER])

            # ---- projected coords stats (small [8, NPER] pipeline) ----
            raw_ps = ps.tile([2 * M, NPER], F32, tag="rawp")
            for q in range(0, NPER, 512):
                nc.tensor.matmul(raw_ps[:, q:q + 512], wmat[:],
                                 xyzT[:, q:q + 512], start=True, stop=True)
            raw = prep.tile([2 * M, NPER], F32)
            nc.scalar.activation(raw[:], raw_ps[:], AF.Identity,
                                 bias=bias8[:], scale=1.0)
            rmax = prep.tile([2 * M, 1], F32)
            rmin = prep.tile([2 * M, 1], F32)
            nc.vector.tensor_reduce(rmax[:], raw[:], axis=mybir.AxisListType.X, op=A.max)
            nc.vector.tensor_reduce(rmin[:], raw[:], axis=mybir.AxisListType.X, op=A.min)
            cen = prep.tile([2 * M, 1], F32)
            nc.vector.tensor_tensor(cen[:], rmax[:], rmin[:], op=A.add)
            nc.vector.tensor_scalar_mul(cen[:], cen[:], 0.5)
            sp = prep.tile([2 * M, 1], F32)
            nc.vector.tensor_tensor(sp[:], rmax[:], rmin[:], op=A.subtract)
            nc.vector.tensor_scalar_max(sp[:], sp[:], 1e-5)
            rec = prep.tile([2 * M, 1], F32)
            nc.vector.reciprocal(rec[:], sp[:])
            a8 = prep.tile([2 * M, 1], F32)
            nc.vector.tensor_scalar_mul(a8[:], rec[:], 38.4)
            # b2 = a8*(bias8 - cen) + 24
            u8 = prep.tile([2 * M, 1], F32)
            nc.vector.tensor_tensor(u8[:], bias8[:], cen[:], op=A.subtract)
            b2 = prep.tile([2 * M, 1], F32)
            nc.vector.tensor_tensor(b2[:], a8[:], u8[:], op=A.mult)
            nc.vector.tensor_scalar_add(b2[:], b2[:], 24.0)

            # broadcast a8/b2 to [128, 16] via transpose + ones matmul
            abT_ps = ps.tile([1, 16], F32, tag="pp")
            nc.tensor.transpose(abT_ps[0:1, 0:8], a8[:], ident[0:8, 0:8])
            nc.tensor.transpose(abT_ps[0:1, 8:16], b2[:], ident[0:8, 0:8])
            abrow = prep.tile([1, 16], F32)
            nc.vector.tensor_copy(abrow[:], abT_ps[:])
            abB_ps = ps.tile([128, 16], F32, tag="pp")
            nc.tensor.matmul(abB_ps[:, 0:8], ones1[:], abrow[0:1, 0:8],
                             start=True, stop=True)
            nc.tensor.matmul(abB_ps[:, 8:16], ones1[:], abrow[0:1, 8:16],
                             start=True, stop=True)
            abB = prep.tile([128, 16], F32)
            nc.scalar.copy(abB[:], abB_ps[:])

            # ---- per-row broadcast coords: matmul with replicated weights,
            # normalization fused into the PSUM->SBUF copy ----
            rows = []
            for r in range(2 * M):
                pb_ps = ps2.tile([128, NPER], F32, tag="pb_ps")
                for q in range(0, NPER, 512):
                    nc.tensor.matmul(pb_ps[:, q:q + 512],
                                     wrep[:, r * 128:(r + 1) * 128],
                                     xyzT[:, q:q + 512], start=True, stop=True)
                prow = big.tile([128, NPER], F32, tag=f"prow{r}")
                nc.scalar.activation(prow[:], pb_ps[:], AF.Identity,
                                     bias=abB[:, 8 + r:9 + r], scale=abB[:, r:r + 1])
                rows.append(prow)

            # ---- main loop ----
            fsdcols = big.tile([128, M * NT], F32)
            cntcols = big.tile([128, M * NT], F32)
            junk = big.tile([128, NPER], BF16)
            for m in range(M):
                for t in range(NT):
                    col = m * NT + t
                    dx2 = lp.tile([128, NPER], F32, tag="dx2")
                    dy2 = lp.tile([128, NPER], F32, tag="dy2")
                    nc.scalar.activation(dx2[:], rows[2 * m][:], AF.Square,
                                         bias=ngx[:, t:t + 1], scale=1.0)
                    nc.scalar.activation(dy2[:], rows[2 * m + 1][:], AF.Square,
                                         bias=ngy[:, t:t + 1], scale=1.0)
                    t4 = lp.tile([128, NPER], F32, tag="t4")
                    nc.scalar.activation(t4[:], dy2[:], AF.Identity,
                                         bias=c4b[:], scale=-1.0)
                    w = lp.tile([128, NPER], BF16, tag="w")
                    nc.vector.scalar_tensor_tensor(
                        w[:], dx2[:], 1.0, t4[:], op0=A.mult, op1=A.is_lt)
                    satc = lp.tile([128, NPER], BF16, tag="satc")
                    nc.vector.tensor_tensor_scan(
                        satc[:], w[:], c16t[:], 0.0, op0=A.add, op1=A.min)
                    nc.vector.scalar_tensor_tensor(
                        junk[:], satc[:], 1.0, eab[:], op0=A.mult, op1=A.mult,
                        accum_out=fsdcols[:, col:col + 1])
                    nc.vector.tensor_copy(
                        cntcols[:, col:col + 1], satc[:, NPER - 1:NPER])

            # ---- tail ----
            occ = prep.tile([128, M * NT], F32)
            nc.vector.tensor_scalar_max(occ[:], cntcols[:], 1.0)
            rocc = prep.tile([128, M * NT], F32)
            nc.vector.reciprocal(rocc[:], occ[:])
            dlog = prep.tile([128, M * NT], F32)
            nc.vector.tensor_tensor(dlog[:], fsdcols[:], rocc[:], op=A.mult)
            p1 = prep.tile([128, M * NT], F32)
            nc.scalar.activation(p1[:], dlog[:], AF.Sigmoid, bias=czero[:], scale=1.0)
            nz = prep.tile([128, M * NT], F32)
            nc.vector.tensor_scalar(nz[:], cntcols[:], 0.0, None, op0=A.is_gt)
            outt = prep.tile([128, M * NT], F32)
            nc.vector.scalar_tensor_tensor(
                outt[:], p1[:], 255.0, nz[:], op0=A.mult, op1=A.mult)
            nc.sync.dma_start(
                d_out[:].rearrange("m (t p) -> p m t", p=128),
                outt[:].rearrange("p (m t) -> p m t", m=M))
    nc.compile()
    return nc


def host_inputs(xyz, features, theta, phi):
    """Build per-core input maps. xyz (1,N,3) f32, features (1,N,C) f32."""
    theta = np.asarray(theta, np.float32)
    phi = np.asarray(phi, np.float32)
    sint, cost = np.sin(theta), np.cos(theta)
    sinp, cosp = np.sin(phi), np.cos(phi)
    U = np.stack([-sint, cost, np.zeros_like(theta)], -1)      # (M,3)
    V = np.stack([cost * sinp, sint * sinp, cosp], -1)         # (M,3)
    center = np.stack([cost * cosp, sint * cosp, sinp], -1)    # (M,3)
    wmat = np.zeros((3, 2 * M), np.float32)
    bias8 = np.zeros((2 * M, 1), np.float32)
    for m in range(M):
        wmat[:, 2 * m] = U[m]
        wmat[:, 2 * m + 1] = V[m]
        bias8[2 * m, 0] = -np.dot(center[m], U[m])
        bias8[2 * m + 1, 0] = -np.dot(center[m], V[m])

    s = np.arange(S)
    p = s % 128
    t = s // 128
    ngx = np.zeros((128, NT), np.float32)
    ngy = np.zeros((128, NT), np.float32)
    ngx[p, t] = -(s // RES).astype(np.float32)
    ngy[p, t] = -(s % RES).astype(np.float32)

    ident = np.eye(128, dtype=np.float32)
    ones1 = np.ones((1, 128), np.float32)
    eye8rep = np.zeros((NCHUNK, NCHUNK * 128), np.float32)
    for c in range(NCHUNK):
        eye8rep[c, c * 128:(c + 1) * 128] = 1.0
    wrep = np.repeat(wmat, 128, axis=1)  # (3, 8*128), col r*128+p = wmat[:, r]

    xyz = np.asarray(xyz, np.float32)[0]          # (N,3)
    features = np.asarray(features, np.float32)[0]  # (N,C)
    in_maps = []
    for c in range(K):
        pts = xyz[c * NPER:(c + 1) * NPER]        # (1024,3)
        ft = features[c * NPER:(c + 1) * NPER]    # (1024,20)
        featsR = np.ascontiguousarray(
            ft.reshape(NCHUNK, 128, C).transpose(1, 0, 2).reshape(128, NCHUNK * C))
        in_maps.append({
            "xyzT": np.ascontiguousarray(pts.T),
            "featsR": featsR,
            "wmat": wmat, "bias8": bias8,
            "ngx": ngx, "ngy": ngy,
            "ones1": ones1, "eye8rep": eye8rep, "wrep": wrep, "ident": ident,
        })
    return in_maps


_CACHE = {}


def kernel(xyz, features, proposals, res, theta, phi):
    if "nc" not in _CACHE:
        _CACHE["nc"] = build_program()
    nc = _CACHE["nc"]
    in_maps = host_inputs(xyz, features, theta, phi)
    res_k = run_bass_kernel_spmd(nc, in_maps, list(range(K)))
    mask1 = np.stack([np.asarray(res_k.results[c]["out"]) for c in range(K)])
    mask1 = mask1.reshape(K * M, 1, RES, RES)
    out = np.broadcast_to(mask1, (K * M, 3, RES, RES))
    return np.ascontiguousarray(out.astype(np.float32))


# revision 22
# speedup vs baseline: 1.2094x; 1.0020x over previous
"""Trainium2 Bass kernel for nn_Point2Mask (retrieval_knn).

Full inputs in, full output out. Sharding: data-parallel over the K=8
instance axis — core c handles instance c (its 1024 points) across all
M=4 views. Each core rasterizes the 48x48 ball-query mask for its 4
(instance, view) pairs.

Device algorithm per core:
  - project points to 2D per view (PE matmul), normalize via min/max
  - per-point feature top2-gap df (segmented reduces)
  - broadcast coords/df rows to 128 partitions via ones-matmul (PE)
  - main loop over 4 views x 18 pixel-tiles of [128 pixels, 1024 points]:
      ACT: dx2 = Square(px - gx), dy2 = Square(py - gy), t4 = 4 - dy2
      DVE: within = dx2 < t4 (bf16), inclusive cumsum (tensor_tensor_scan),
           sel = (csum <= 16) & within, fsd = sum(sel * df) via accum_out,
           count = min(csum_last, 16)
  - tail: p1 = sigmoid(fsd / max(count,1)) * 255, zeroed where count==0
"""
import numpy as np

import concourse.bass as bass
import concourse.bacc as bacc
import concourse.mybir as mybir
from concourse import tile
from concourse.bass_utils import run_bass_kernel_spmd

K = 8
M = 4
N = 8192
NPER = N // K          # 1024 points per instance
C = 20
RES = 48
S = RES * RES          # 2304 pixels
NT = S // 128          # 18 pixel tiles
NCHUNK = NPER // 128   # 8 point chunks
R2 = 4.0               # radius^2
NS = 16.0              # nsample

F32 = mybir.dt.float32
BF16 = mybir.dt.bfloat16
A = mybir.AluOpType
AF = mybir.ActivationFunctionType


def build_program():
    nc = bacc.Bacc("TRN2", target_bir_lowering=False, debug=True)
    d_xyzT = nc.dram_tensor("xyzT", [3, NPER], F32, kind="ExternalInput")
    d_feats = nc.dram_tensor("featsR", [128, NCHUNK * C], F32, kind="ExternalInput")
    d_wmat = nc.dram_tensor("wmat", [3, 2 * M], F32, kind="ExternalInput")
    d_bias8 = nc.dram_tensor("bias8", [2 * M, 1], F32, kind="ExternalInput")
    d_ngx = nc.dram_tensor("ngx", [128, NT], F32, kind="ExternalInput")
    d_ngy = nc.dram_tensor("ngy", [128, NT], F32, kind="ExternalInput")
    d_ones1 = nc.dram_tensor("ones1", [1, 128], F32, kind="ExternalInput")
    d_eye8 = nc.dram_tensor("eye8rep", [NCHUNK, NCHUNK * 128], F32, kind="ExternalInput")
    d_wrep = nc.dram_tensor("wrep", [3, 2 * M * 128], F32, kind="ExternalInput")
    d_ident = nc.dram_tensor("ident", [128, 128], F32, kind="ExternalInput")
    d_out = nc.dram_tensor("out", [M, S], F32, kind="ExternalOutput")

    with tile.TileContext(nc) as tc:
        with (
            tc.tile_pool(name="cst", bufs=1) as cst,
            tc.tile_pool(name="prep", bufs=1) as prep,
            tc.tile_pool(name="big", bufs=1) as big,
            tc.tile_pool(name="loop", bufs=2) as lp,
            tc.tile_pool(name="ps", bufs=1, space="PSUM") as ps,
            tc.tile_pool(name="ps2", bufs=2, space="PSUM") as ps2,
        ):
            # ---- load constants/inputs ----
            xyzT = cst.tile([3, NPER], F32)
            feats = cst.tile([128, NCHUNK * C], F32)
            wmat = cst.tile([3, 2 * M], F32)
            bias8 = cst.tile([2 * M, 1], F32)
            ngx = cst.tile([128, NT], F32)
            ngy = cst.tile([128, NT], F32)
            ones1 = cst.tile([1, 128], F32)
            eye8 = cst.tile([NCHUNK, NCHUNK * 128], F32)
            wrep = cst.tile([3, 2 * M * 128], F32)
            ident = cst.tile([128, 128], F32)
            for t, d in [(xyzT, d_xyzT), (feats, d_feats), (wmat, d_wmat),
                         (bias8, d_bias8), (ngx, d_ngx), (ngy, d_ngy),
                         (ones1, d_ones1), (eye8, d_eye8), (wrep, d_wrep),
                         (ident, d_ident)]:
                nc.sync.dma_start(t[:], d[:])

            c4b = cst.tile([128, 1], F32)
            nc.vector.memset(c4b[:], R2)
            czero = cst.tile([128, 1], F32)
            nc.vector.memset(czero[:], 0.0)
            c16t = cst.tile([128, NPER], BF16)
            nc.vector.memset(c16t[:], NS)

            # ---- df = top1 - top2 of features per point ----
            fview = feats[:].rearrange("p (c k) -> p c k", c=NCHUNK)
            m1 = prep.tile([128, NCHUNK], F32)
            nc.vector.tensor_reduce(
                m1[:].rearrange("p (c o) -> p c o", o=1), fview,
                axis=mybir.AxisListType.X, op=A.max)
            h = prep.tile([128, NCHUNK * C], F32)
            for c in range(NCHUNK):
                nc.vector.tensor_scalar(
                    h[:, c * C:(c + 1) * C], feats[:, c * C:(c + 1) * C],
                    m1[:, c:c + 1], None, op0=A.subtract)
            hmask = prep.tile([128, NCHUNK * C], F32)
            nc.vector.tensor_scalar(hmask[:], h[:], 0.0, None, op0=A.is_ge)
            hm = prep.tile([128, NCHUNK * C], F32)
            nc.vector.scalar_tensor_tensor(
                hm[:], hmask[:], -1e9, h[:], op0=A.mult, op1=A.add)
            m2n = prep.tile([128, NCHUNK], F32)
            nc.vector.tensor_reduce(
                m2n[:].rearrange("p (c o) -> p c o", o=1),
                hm[:].rearrange("p (c k) -> p c k", c=NCHUNK),
                axis=mybir.AxisListType.X, op=A.max)
            df8 = prep.tile([128, NCHUNK], F32)
            nc.vector.tensor_scalar_mul(df8[:], m2n[:], -1.0)

            # transpose df8 -> [NCHUNK, 128], then broadcast chunks via
            # one-hot matmuls: dfb[p, c*128+q] = dfrowS[c, q]
            dfrow_ps = ps.tile([NCHUNK, 128], F32, tag="pp")
            nc.tensor.transpose(dfrow_ps[:], df8[:], ident[:])
            dfrowS = prep.tile([NCHUNK, 128], F32)
            nc.scalar.copy(dfrowS[:], dfrow_ps[:])
            dfb_ps = ps.tile([128, NPER], F32, tag="pp")
            for c in range(NCHUNK):
                nc.tensor.matmul(dfb_ps[:, c * 128:(c + 1) * 128],
                                 eye8[:, c * 128:(c + 1) * 128], dfrowS[:],
                                 start=True, stop=True)
            dfbf = big.tile([128, NPER], F32)
            nc.scalar.copy(dfbf[:], dfb_ps[:])
            eab = big.tile([128, NPER], F32)
            nc.vector.tensor_tensor(eab[:, 0:NPER - 1], dfbf[:, 0:NPER - 1],
                                    dfbf[:, 1:NPER], op=A.subtract)
            nc.vector.tensor_copy(eab[:, NPER - 1:NPER], dfbf[:, NPER - 1:NPER])

            # ---- projected coords stats (small [8, NPER] pipeline) ----
            raw_ps = ps.tile([2 * M, NPER], F32, tag="rawp")
            for q in range(0, NPER, 512):
                nc.tensor.matmul(raw_ps[:, q:q + 512], wmat[:],
                                 xyzT[:, q:q + 512], start=True, stop=True)
            raw = prep.tile([2 * M, NPER], F32)
            nc.scalar.activation(raw[:], raw_ps[:], AF.Identity,
                                 bias=bias8[:], scale=1.0)
            rmax = prep.tile([2 * M, 1], F32)
            rmin = prep.tile([2 * M, 1], F32)
            nc.vector.tensor_reduce(rmax[:], raw[:], axis=mybir.AxisListType.X, op=A.max)
            nc.vector.tensor_reduce(rmin[:], raw[:], axis=mybir.AxisListType.X, op=A.min)
            cen = prep.tile([2 * M, 1], F32)
            nc.vector.tensor_tensor(cen[:], rmax[:], rmin[:], op=A.add)
            nc.vector.tensor_scalar_mul(cen[:], cen[:], 0.5)
            sp = prep.tile([2 * M, 1], F32)
            nc.vector.tensor_tensor(sp[:], rmax[:], rmin[:], op=A.subtract)
            nc.vector.tensor_scalar_max(sp[:], sp[:], 1e-5)
            rec = prep.tile([2 * M, 1], F32)
            nc.vector.reciprocal(rec[:], sp[:])
            a8 = prep.tile([2 * M, 1], F32)
            nc.vector.tensor_scalar_mul(a8[:], rec[:], 38.4)
            # b2 = a8*(bias8 - cen) + 24
            u8 = prep.tile([2 * M, 1], F32)
            nc.vector.tensor_tensor(u8[:], bias8[:], cen[:], op=A.subtract)
            b2 = prep.tile([2 * M, 1], F32)
            nc.vector.tensor_tensor(b2[:], a8[:], u8[:], op=A.mult)
            nc.vector.tensor_scalar_add(b2[:], b2[:], 24.0)

            # broadcast a8/b2 to [128, 16] via transpose + ones matmul
            abT_ps = ps.tile([1, 16], F32, tag="pp")
            nc.tensor.transpose(abT_ps[0:1, 0:8], a8[:], ident[0:8, 0:8])
            nc.tensor.transpose(abT_ps[0:1, 8:16], b2[:], ident[0:8, 0:8])
            abrow = prep.tile([1, 16], F32)
            nc.vector.tensor_copy(abrow[:], abT_ps[:])
            abB_ps = ps.tile([128, 16], F32, tag="pp")
            nc.tensor.matmul(abB_ps[:, 0:8], ones1[:], abrow[0:1, 0:8],
                             start=True, stop=True)
            nc.tensor.matmul(abB_ps[:, 8:16], ones1[:], abrow[0:1, 8:16],
                             start=True, stop=True)
            abB = prep.tile([128, 16], F32)
            nc.scalar.copy(abB[:], abB_ps[:])

            # ---- per-row broadcast coords: matmul with replicated weights,
            # normalization fused into the PSUM->SBUF copy ----
            rows = []
            for r in range(2 * M):
                pb_ps = ps2.tile([128, NPER], F32, tag="pb_ps")
                for q in range(0, NPER, 512):
                    nc.tensor.matmul(pb_ps[:, q:q + 512],
                                     wrep[:, r * 128:(r + 1) * 128],
                                     xyzT[:, q:q + 512], start=True, stop=True)
                prow = big.tile([128, NPER], F32, tag=f"prow{r}")
                nc.scalar.activation(prow[:], pb_ps[:], AF.Identity,
                                     bias=abB[:, 8 + r:9 + r], scale=abB[:, r:r + 1])
                rows.append(prow)

            # ---- main loop ----
            fsdcols = big.tile([128, M * NT], F32)
            cntcols = big.tile([128, M * NT], F32)
            junk = big.tile([128, NPER], BF16)
            for m in range(M):
                for t in range(NT):
                    col = m * NT + t
                    dx2 = lp.tile([128, NPER], F32, tag="dx2")
                    dy2 = lp.tile([128, NPER], F32, tag="dy2")
                    nc.scalar.activation(dx2[:], rows[2 * m][:], AF.Square,
                                         bias=ngx[:, t:t + 1], scale=1.0)
                    nc.scalar.activation(dy2[:], rows[2 * m + 1][:], AF.Square,
                                         bias=ngy[:, t:t + 1], scale=1.0)
                    t4 = lp.tile([128, NPER], F32, tag="t4")
                    nc.scalar.activation(t4[:], dy2[:], AF.Identity,
                                         bias=c4b[:], scale=-1.0)
                    w = lp.tile([128, NPER], BF16, tag="w")
                    nc.vector.scalar_tensor_tensor(
                        w[:], dx2[:], 1.0, t4[:], op0=A.mult, op1=A.is_lt,
                        accum_out=cntcols[:, col:col + 1])
                    satc = lp.tile([128, NPER], BF16, tag="satc")
                    nc.vector.tensor_tensor_scan(
                        satc[:], w[:], c16t[:], 0.0, op0=A.add, op1=A.min)
                    nc.vector.scalar_tensor_tensor(
                        junk[:], satc[:], 1.0, eab[:], op0=A.mult, op1=A.mult,
                        accum_out=fsdcols[:, col:col + 1])

            # ---- tail ----
            occ = prep.tile([128, M * NT], F32)
            nc.vector.tensor_scalar(occ[:], cntcols[:], 16.0, 1.0,
                                    op0=A.min, op1=A.max)
            rocc = prep.tile([128, M * NT], F32)
            nc.vector.reciprocal(rocc[:], occ[:])
            dlog = prep.tile([128, M * NT], F32)
            nc.vector.tensor_tensor(dlog[:], fsdcols[:], rocc[:], op=A.mult)
            p1 = prep.tile([128, M * NT], F32)
            nc.scalar.activation(p1[:], dlog[:], AF.Sigmoid, bias=czero[:], scale=1.0)
            nz = prep.tile([128, M * NT], F32)
            nc.vector.tensor_scalar(nz[:], cntcols[:], 0.0, None, op0=A.is_gt)
            outt = prep.tile([128, M * NT], F32)
            nc.vector.scalar_tensor_tensor(
                outt[:], p1[:], 255.0, nz[:], op0=A.mult, op1=A.mult)
            nc.sync.dma_start(
                d_out[:].rearrange("m (t p) -> p m t", p=128),
                outt[:].rearrange("p (m t) -> p m t", m=M))
    nc.compile()
    return nc


def host_inputs(xyz, features, theta, phi):
    """Build per-core input maps. xyz (1,N,3) f32, features (1,N,C) f32."""
    theta = np.asarray(theta, np.float32)
    phi = np.asarray(phi, np.float32)
    sint, cost = np.sin(theta), np.cos(theta)
    sinp, cosp = np.sin(phi), np.cos(phi)
    U = np.stack([-sint, cost, np.zeros_like(theta)], -1)      # (M,3)
    V = np.stack([cost * sinp, sint * sinp, cosp], -1)         # (M,3)
    center = np.stack([cost * cosp, sint * cosp, sinp], -1)    # (M,3)
    wmat = np.zeros((3, 2 * M), np.float32)
    bias8 = np.zeros((2 * M, 1), np.float32)
    for m in range(M):
        wmat[:, 2 * m] = U[m]
        wmat[:, 2 * m + 1] = V[m]
        bias8[2 * m, 0] = -np.dot(center[m], U[m])
        bias8[2 * m + 1, 0] = -np.dot(center[m], V[m])

    s = np.arange(S)
    p = s % 128
    t = s // 128
    ngx = np.zeros((128, NT), np.float32)
    ngy = np.zeros((128, NT), np.float32)
    ngx[p, t] = -(s // RES).astype(np.float32)
    ngy[p, t] = -(s % RES).astype(np.float32)

    ident = np.eye(128, dtype=np.float32)
    ones1 = np.ones((1, 128), np.float32)
    eye8rep = np.zeros((NCHUNK, NCHUNK * 128), np.float32)
    for c in range(NCHUNK):
        eye8rep[c, c * 128:(c + 1) * 128] = 1.0
    wrep = np.repeat(wmat, 128, axis=1)  # (3, 8*128), col r*128+p = wmat[:, r]

    xyz = np.asarray(xyz, np.float32)[0]          # (N,3)
    features = np.asarray(features, np.float32)[0]  # (N,C)
    in_maps = []
    for c in range(K):
        pts = xyz[c * NPER:(c + 1) * NPER]        # (1024,3)
        ft = features[c * NPER:(c + 1) * NPER]    # (1024,20)
        featsR = np.ascontiguousarray(
            ft.reshape(NCHUNK, 128, C).transpose(1, 0, 2).reshape(128, NCHUNK * C))
        in_maps.append({
            "xyzT": np.ascontiguousarray(pts.T),
            "featsR": featsR,
            "wmat": wmat, "bias8": bias8,
            "ngx": ngx, "ngy": ngy,
            "ones1": ones1, "eye8rep": eye8rep, "wrep": wrep, "ident": ident,
        })
    return in_maps


_CACHE = {}


def kernel(xyz, features, proposals, res, theta, phi):
    if "nc" not in _CACHE:
        _CACHE["nc"] = build_program()
    nc = _CACHE["nc"]
    in_maps = host_inputs(xyz, features, theta, phi)
    res_k = run_bass_kernel_spmd(nc, in_maps, list(range(K)))
    mask1 = np.stack([np.asarray(res_k.results[c]["out"]) for c in range(K)])
    mask1 = mask1.reshape(K * M, 1, RES, RES)
    out = np.broadcast_to(mask1, (K * M, 3, RES, RES))
    return np.ascontiguousarray(out.astype(np.float32))
